# revision 24
# baseline (speedup 1.0000x reference)
"""BinaryExceptOutliersLinear on 8 Trainium2 NeuronCores — fp8 DoubleRow version.

Reference computation:
    w_bin = where(|w - mean(w)| > std(w), w, sign(w))   (mean/std over all of w, ddof=1)
    out[b,s,o] = sum_k x[b,s,k] * w_bin[o,k] + bias[o]

Strategy (data-parallel over tokens):
  - Batch dim B=8 sharded across 8 cores (2048 tokens each); every core gets
    the full weight (host-side pre-transposed to wT=[d_in, d_out] so the
    contraction dim lands on SBUF partitions with no PE transposes for w) and
    computes its tokens' full output row-block.  No collectives.
  - Thresholds (mean/std of w) are computed host-side bit-exactly as in the
    reference (jax CPU fp32); the binarize (clamp/compare/select + sign) runs
    on-device against the exact lower/upper scalars:
      Pool: clamp   DVE: not_equal mask, copy_predicated   ACT: Sign
    writing fp8e4m3 directly (signs are exact in fp8; outlier values are tiny
    so their fp8 rounding is negligible).  Chunks are processed in ks-pairs
    ([128, 2, 512] tiles) to halve per-instruction overheads.
  - Matmul runs in fp8e4m3 with MatmulPerfMode.DoubleRow (2 k-subtiles per
    instruction at 0.5 cycles/row per the TRN2 cost model) accumulating in
    fp32 PSUM.  A single fp8 x limb fails the 2e-2 gate (measured 2.7e-2), so
    x is split into limbs hi=fp8(x), lo=fp8(x-hi); the lo correction is
    applied on the first LO_KP of 16 k-pair groups (LO_KP=8 measures 1.71e-2,
    bit-identical between HW and the numpy model).
  - The tile pipeline splits every matmul into Ldweights+Matmult and the cost
    model charges each Ldweights ~105ns of serialized PE-sequencer time,
    which would gate the kernel.  The steady-state loop therefore orders
    matmuls j-outer/tt-inner so one weight load feeds 8 matmuls (4 token
    tiles x hi+lo limbs, 4 concurrent PSUM accumulation groups) and a
    post-compile pass deletes the now-redundant consecutive Ldweights (same
    weights AP, no sync info, no intervening PE state change).
  - x ships from the host in bf16 (halves the x DMA; the limb split absorbs
    the rounding), is PE-transposed in bf16 (1 cyc/row); the PSUM->SBUF
    copyback on ACT is the hi-limb cast, one DVE scalar_tensor_tensor forms
    the lo limb.  Output is written transposed [d_out, t] in bf16 and the
    host casts back to f32.
"""

import os
import sys

import numpy as np

for _p in ("/opt/trn_rl_repo", "/opt/pypackages"):
    if os.path.isdir(_p) and _p not in sys.path:
        sys.path.append(_p)

P = 128
B, S, D_IN, D_OUT = 8, 2048, 4096, 4096
N_CORES = 8
T = (B * S) // N_CORES  # tokens per core = 2048
KSUB = D_IN // P        # 32 k-subtiles
KP = KSUB // 2          # 16 k-subtile pairs (DoubleRow granularity)
LO_KP = 8               # k-pairs receiving the lo-limb correction
TT = 512                # token tile (psum width)
T_TILES = T // TT       # 4
OB = 512                # o-block width (w binarize granularity)
O_BLOCKS = D_OUT // OB  # 8
OT_PER = OB // P        # 4 o-tiles per block


def dedupe_ldweights(nc):
    """Delete Ldweights that reload the exact weights already in the PE array.

    Safe iff: previous surviving Ldweights has an identical weights AP, the
    candidate carries no sync info, and no other PE-array-state-changing
    instruction (transpose/self-loading matmul of different weights) sits in
    between.  Matmults between are fine: whether or not they self-load, the
    weights they use are identical by construction.
    """
    import concourse.mybir as mybir

    def sig(ap):
        mls = getattr(ap, "memorylocations", None)
        name = None
        try:
            name = ap.memloc_name
        except Exception:
            pass
        if name is None:
            name = str(getattr(ap, "name", "")) or repr(ap)[:80]
        return (name, ap.offset, tuple(tuple(d) for d in ap.ap))

    removed = 0
    for blk in nc.m.functions[0].blocks:
        insts = list(blk.instructions)
        keep = []
        last_w = None
        for inst in insts:
            if isinstance(inst, mybir.InstLdweights):
                si = inst.sync_info
                clean = si is None or (len(si.on_wait) == 0 and len(si.on_update) == 0)
                s = sig(inst.ins[0])
                if clean and last_w == s:
                    removed += 1
                    continue
                last_w = s
                keep.append(inst)
            elif isinstance(inst, mybir.InstMatmult):
                if inst.is_transpose:
                    last_w = None
                elif last_w is not None and len(inst.ins) >= 2:
                    # conservative: a matmul streaming different weights than
                    # the last load invalidates the loaded-weights tracking
                    if sig(inst.ins[1]) != last_w:
                        last_w = None
                keep.append(inst)
            else:
                if getattr(inst, "engine", None) == mybir.EngineType.PE and not isinstance(
                    inst, (mybir.InstEventSemaphore,)
                ):
                    last_w = None
                keep.append(inst)
        if removed:
            while len(blk.instructions):
                blk.instructions.pop()
            for inst in keep:
                blk.instructions.append(inst)
    return removed


def build_program(repeats=1, lo_kp=LO_KP, dedupe=True):
    """Single-core Bass/Tile program (same program on all cores)."""
    import concourse.mybir as mybir
    import concourse.tile as tile
    from concourse import bacc
    from concourse.masks import make_identity

    F32 = mybir.dt.float32
    BF16 = mybir.dt.bfloat16
    FP8 = mybir.dt.float8e4
    AF = mybir.ActivationFunctionType
    ALU = mybir.AluOpType
    DR = mybir.MatmulPerfMode.DoubleRow

    LK = lo_kp
    LO_KS = 2 * LK          # k-subtiles covered by the lo limb

    nc = bacc.Bacc(
        "TRN2",
        target_bir_lowering=False,
        debug=False,
        enable_asserts=False,
        num_devices=1,
    )

    x = nc.dram_tensor("x", [T, D_IN], BF16, kind="ExternalInput").ap()
    wT = nc.dram_tensor("wT", [D_IN, D_OUT], F32, kind="ExternalInput").ap()
    bias = nc.dram_tensor("bias", [D_OUT], F32, kind="ExternalInput").ap()
    thr = nc.dram_tensor("thr", [P, 2], F32, kind="ExternalInput").ap()
    outT = nc.dram_tensor("outT", [D_OUT, T], BF16, kind="ExternalOutput").ap()

    with tile.TileContext(nc) as tc:
      for _rep in range(repeats):
        with (
            tc.tile_pool(name="const", bufs=1) as const,
            tc.tile_pool(name="psum_acc", bufs=6, space="PSUM") as psum_acc,
            tc.tile_pool(name="psum_t", bufs=2, space="PSUM") as psum_t,
            tc.tile_pool(name="wraw", bufs=3) as wraw_pool,
            tc.tile_pool(name="wclamp", bufs=2) as wclamp_pool,
            tc.tile_pool(name="wmask", bufs=2) as wmask_pool,
            tc.tile_pool(name="wt", bufs=2) as wt_pool,
            tc.tile_pool(name="osb", bufs=4) as osb_pool,
        ):
            ident = const.tile([P, P], BF16)
            make_identity(nc, ident)

            bias_sb = const.tile([P, D_OUT // P], F32)
            nc.sync.dma_start(bias_sb, bias.rearrange("(o p) -> p o", p=P))
            thr_sb = const.tile([P, 2], F32)
            nc.sync.dma_start(thr_sb, thr)
            lower = thr_sb[:, 0:1]
            upper = thr_sb[:, 1:2]

            # x^T fp8 limbs resident in SBUF
            xT_hi = const.tile([P, KSUB, T], FP8)
            xT_lo = const.tile([P, LO_KS, T], FP8)

            def emit_w_pair(ob, kp, wt_tile):
                """Binarize k-subtile pair (2*kp, 2*kp+1) of o-block ob."""
                ks = 2 * kp
                wraw = wraw_pool.tile([P, 2, OB], F32, name="wraw", tag="wraw")
                nc.sync.dma_start(
                    wraw,
                    wT[ks * P : (ks + 2) * P, ob * OB : (ob + 1) * OB].rearrange(
                        "(two p) o -> p two o", p=P
                    ),
                )
                wc = wclamp_pool.tile([P, 2, OB], F32, name="wc", tag="wc")
                nc.gpsimd.tensor_scalar(wc, wraw, lower, upper, ALU.max, ALU.min)
                wm = wmask_pool.tile([P, 2, OB], mybir.dt.uint8, name="wm", tag="wm")
                nc.vector.tensor_tensor(wm, wc, wraw, ALU.not_equal)
                dst = wt_tile[:, ks : ks + 2, :]
                nc.scalar.activation(dst, wraw, AF.Sign)
                nc.vector.copy_predicated(dst, wm, wraw)

            def new_wt_tile():
                return wt_pool.tile([P, KSUB, OB], FP8, name="wt", tag="wt")

            def evict(ob, ot, tt, psum):
                col = ob * OT_PER + ot
                osb = osb_pool.tile([P, TT], BF16, name="osb", tag="osb")
                nc.scalar.activation(
                    osb, psum, AF.Identity, bias=bias_sb[:, col : col + 1]
                )
                nc.sync.dma_start(
                    outT[col * P : (col + 1) * P, tt * TT : (tt + 1) * TT], osb
                )

            def emit_mm_group(ob, wt_tile, ot, tt):
                """Single-tt accumulation group (prepass path; no weight reuse)."""
                psum = psum_acc.tile([P, TT], F32, name="acc", tag="acc")
                o0 = ot * P
                t0 = tt * TT
                for j in range(KP):
                    nc.tensor.matmul(
                        psum,
                        wt_tile[:, 2 * j : 2 * j + 2, o0 : o0 + P],
                        xT_hi[:, 2 * j : 2 * j + 2, t0 : t0 + TT],
                        start=(j == 0),
                        stop=False,
                        perf_mode=DR,
                    )
                for j in range(LK):
                    nc.tensor.matmul(
                        psum,
                        wt_tile[:, 2 * j : 2 * j + 2, o0 : o0 + P],
                        xT_lo[:, 2 * j : 2 * j + 2, t0 : t0 + TT],
                        start=False,
                        stop=(j == LK - 1),
                        perf_mode=DR,
                    )
                evict(ob, ot, tt, psum)

            def emit_otile_reuse(ob, wt_tile, ot):
                """j-outer / tt-inner: one weight load serves 4 tts x 2 limbs."""
                o0 = ot * P
                psums = [
                    psum_acc.tile([P, TT], F32, name=f"acc{tt}", tag="acc")
                    for tt in range(T_TILES)
                ]
                for j in range(KP):
                    w_sl = wt_tile[:, 2 * j : 2 * j + 2, o0 : o0 + P]
                    last_j = j == KP - 1
                    for tt in range(T_TILES):
                        nc.tensor.matmul(
                            psums[tt],
                            w_sl,
                            xT_hi[:, 2 * j : 2 * j + 2, tt * TT : (tt + 1) * TT],
                            start=(j == 0),
                            stop=(last_j and j >= LK),
                            perf_mode=DR,
                        )
                    if j < LK:
                        for tt in range(T_TILES):
                            nc.tensor.matmul(
                                psums[tt],
                                w_sl,
                                xT_lo[:, 2 * j : 2 * j + 2, tt * TT : (tt + 1) * TT],
                                start=False,
                                stop=last_j,
                                perf_mode=DR,
                            )
                for tt in range(T_TILES):
                    evict(ob, ot, tt, psums[tt])

            # ---- x prepass interleaved with block-0 w pairs and matmuls.
            # Each tg handles one 512-token tile: DMA 4 panels per h-slice,
            # f32 PE-transpose batched 4-wide into psum, ACT copyback = hi
            # cast, DVE scalar_tensor_tensor = lo limb.
            H = 8
            DH = D_IN // H      # 512
            KS_H = KSUB // H    # 4
            wt0 = new_wt_tile()
            with tc.tile_pool(name="xpre", bufs=8) as xpre:
                for tg in range(T_TILES):
                    for h in range(H):
                        if tg == 0:
                            for kp in range(2 * h, 2 * h + 2):
                                emit_w_pair(0, kp, wt0)
                        xraws = []
                        for pi in range(4):
                            tp = tg * 4 + pi
                            xraw = xpre.tile([P, DH], BF16, name="xraw", tag="xraw")
                            nc.sync.dma_start(
                                xraw, x[tp * P : (tp + 1) * P, h * DH : (h + 1) * DH]
                            )
                            xraws.append(xraw)
                        for kl in range(KS_H):
                            ks = h * KS_H + kl
                            pt = psum_t.tile([P, 4 * P], BF16, name="pt", tag="pt")
                            for pi in range(4):
                                nc.tensor.transpose(
                                    pt[:, pi * P : (pi + 1) * P],
                                    xraws[pi][:, kl * P : (kl + 1) * P],
                                    ident,
                                )
                            tok = tg * TT
                            hi_sl = xT_hi[:, ks, tok : tok + TT]
                            nc.scalar.activation(hi_sl, pt, AF.Copy)
                            if ks < LO_KS:
                                nc.vector.scalar_tensor_tensor(
                                    xT_lo[:, ks, tok : tok + TT],
                                    pt,
                                    1.0,
                                    hi_sl,
                                    ALU.mult,
                                    ALU.subtract,
                                )
                    # block-0 matmuls for completed token tiles (skewed one tg)
                    if tg >= 1:
                        for ot in range(OT_PER):
                            emit_mm_group(0, wt0, ot, tg - 1)

            # block 0's final token tile
            for ot in range(OT_PER):
                emit_mm_group(0, wt0, ot, T_TILES - 1)

            # ---- steady state: block ob's o-tiles (weight-reuse form)
            # interleaved with block (ob+1)'s binarize pairs ----
            wt_cur = wt0
            for ob in range(O_BLOCKS):
                nxt = ob + 1
                wt_nxt = new_wt_tile() if nxt < O_BLOCKS else None
                for ot in range(OT_PER):
                    if wt_nxt is not None:
                        for kp in range(4 * ot, 4 * ot + 4):
                            emit_w_pair(nxt, kp, wt_nxt)
                    if ob > 0:
                        emit_otile_reuse(ob, wt_cur, ot)
                wt_cur = wt_nxt

    nc.compile()
    if dedupe:
        n = dedupe_ldweights(nc)
        if os.environ.get("KERNEL_DEBUG"):
            print(f"dedupe_ldweights removed {n}")
    return nc


def _thresholds(weight):
    """Replicate the reference's threshold computation bit-exactly (jax CPU fp32)."""
    import jax
    import jax.numpy as jnp

    cpu = jax.devices("cpu")[0]
    with jax.default_device(cpu):
        wj = jnp.asarray(weight)
        mean = jnp.mean(wj)
        std = jnp.std(wj, ddof=1)
        lower = np.float32(np.asarray(mean - std))
        upper = np.float32(np.asarray(mean + std))
    return lower, upper


_PROGRAM_CACHE = {}


def make_in_maps(x, weight, bias):
    import ml_dtypes

    x = np.ascontiguousarray(np.asarray(x, dtype=np.float32))
    weight = np.ascontiguousarray(np.asarray(weight, dtype=np.float32))
    bias = np.ascontiguousarray(np.asarray(bias, dtype=np.float32))
    lower, upper = _thresholds(weight)
    thr = np.tile(np.array([[lower, upper]], dtype=np.float32), (P, 1))
    wTt = np.ascontiguousarray(weight.T)  # [d_in, d_out]
    # x ships in bf16: the on-device hi/lo fp8 limb split absorbs the
    # rounding (measured absmax err 5.471 vs the f32-shipped 5.494)
    x_sh = x.reshape(N_CORES, T, D_IN).astype(ml_dtypes.bfloat16)
    return [
        {"x": x_sh[i], "wT": wTt, "bias": bias, "thr": thr}
        for i in range(N_CORES)
    ]


def unshard_output(results):
    out = np.empty((N_CORES, T, D_OUT), dtype=np.float32)
    for i in range(N_CORES):
        out[i] = np.asarray(results[i]["outT"]).astype(np.float32).T
    return out.reshape(B, S, D_OUT)


def kernel(x, weight, bias):
    from concourse.bass_utils import run_bass_kernel_spmd

    assert x.shape == (B, S, D_IN) and weight.shape == (D_OUT, D_IN)
    in_maps = make_in_maps(x, weight, bias)
    if "full" not in _PROGRAM_CACHE:
        _PROGRAM_CACHE["full"] = build_program()
    nc = _PROGRAM_CACHE["full"]
    res = run_bass_kernel_spmd(nc, in_maps, core_ids=list(range(N_CORES)))
    return unshard_output(res.results)
